# revision 15
# baseline (speedup 1.0000x reference)
"""Trainium2 Bass kernel for nn_ContextQueryAttentionLayer (v5, bf16).

Math: with B,N,M,D = 32,1024,256,128 the reference's gather index collapses:
  idx[i,j] = (i*M + j) % N = 256*(i%4) + j
so S (b,n,m) has only 4 distinct rows per batch: S[b,i,:] = t[b, i%4, :],
  t[r,j] = q_j.w_q + sum_d (q_{j,d} w_m_d + w_c_d) c_{256r+j,d}
Both softmaxes, C2Q, SM (4x4/batch) and Q2C collapse to rank-4-per-batch:
  out[b,n] = [ctx_n, C2Q[n%4], ctx_n*C2Q[n%4], ctx_n*Q2C[n%4]]

Device computes (bf16 compute / fp32 accum): t, both softmaxes, C2Q, SM4,
CS (class column sums), Q2C, and the two dense products ctx*C2Q[n%4] and
ctx*Q2C[n%4]. The host (pure layout/assembly): shards batches 4-per-core,
pre-permutes inputs to the on-chip layout, precomputes the query-side prep
qwc = q*w_m + w_c and sq = q.w_q, emits section 0 (= the input), broadcasts
the device's C2Q into section 1, and un-permutes the device's product
tensors into sections 2 and 3.

On-chip layout: row n = 256r + 128h + p at partition p, h-major block
(h, r) so every DMA (ctx halves in, product halves out) is fully
contiguous; query row j = 128h + p at partition p. All 4 resident batches
ride in single batched ops; softmax runs in the native [128p, (b,r,h)]
domain (no PE transposes) and is split per-h so the h=0 chain (mul, add
tree, reduce, exp, matmuls) overlaps the h=1 loads/compute. Per-batch
4x4/4x128 matmuls are batched with block-diagonal masking; the soft_c
1/rowsum scale rides in scalar_tensor_tensor (c2q) / ACT scale (q2c).
"""

import numpy as np

B, N, M, D = 32, 1024, 256, 128
NCORES = 8
BPC = B // NCORES  # batches per core

_prog = None


def _build_program():
    import concourse.bacc as bacc
    import concourse.mybir as mybir
    from concourse.tile import TileContext

    fp32 = mybir.dt.float32
    bf16 = mybir.dt.bfloat16
    nc = bacc.Bacc("TRN2", target_bir_lowering=False, name="cqattn5")

    # layouts: ctx[p, h, b, r, d] (n = 256r+128h+p), qwc[p, h, b, d],
    # qry[p, b, h, d] (j = 128h+p)
    ctx_d = nc.dram_tensor("ctx", [128, 2, BPC, 4, 128], bf16, kind="ExternalInput")
    qry_d = nc.dram_tensor("qry", [128, BPC, 2, 128], bf16, kind="ExternalInput")
    qwc_d = nc.dram_tensor("qwc", [128, 2, BPC, 128], bf16, kind="ExternalInput")
    sq_d = nc.dram_tensor("sqv", [128, 2 * BPC], fp32, kind="ExternalInput")
    # [16, 528] fp32: maskC (cols 0:16), maskBD (cols 16:528)
    c16f_d = nc.dram_tensor("c16f", [16, 528], fp32, kind="ExternalInput")
    # [128, 8] bf16: ones128 (col 0), rsel4 (cols 4:8)
    c128b_d = nc.dram_tensor("c128b", [128, 8], bf16, kind="ExternalInput")
    # [16, 144] bf16: rep4x16 (rows 0:4, cols 0:16), b4x16 (cols 16:144)
    c16b_d = nc.dram_tensor("c16b", [16, 144], bf16, kind="ExternalInput")
    prodc_d = nc.dram_tensor(
        "prodC", [128, 2, BPC, 4, 128], bf16, kind="ExternalOutput"
    )
    prodq_d = nc.dram_tensor(
        "prodQ", [128, 2, BPC, 4, 128], bf16, kind="ExternalOutput"
    )
    c2q_d = nc.dram_tensor("c2q", [16, 512], bf16, kind="ExternalOutput")

    Exp = mybir.ActivationFunctionType.Exp
    Copy = mybir.ActivationFunctionType.Copy
    add = mybir.AluOpType.add
    mult = mybir.AluOpType.mult
    X = mybir.AxisListType.X

    with TileContext(nc) as tc:
        with (
            tc.tile_pool(name="io", bufs=1) as io,
            tc.tile_pool(name="work", bufs=1) as work,
            tc.tile_pool(name="small", bufs=1) as small,
            tc.tile_pool(name="outp", bufs=1) as outp,
            tc.tile_pool(name="psum", bufs=1, space="PSUM") as psum,
        ):
            sqv = io.tile([128, 2 * BPC], fp32, tag="sqv", name="sqv")
            c16f = io.tile([16, 528], fp32, tag="c16f", name="c16f")
            c128b = io.tile([128, 8], bf16, tag="c128b", name="c128b")
            c16b = io.tile([16, 144], bf16, tag="c16b", name="c16b")
            ctx_mega = io.tile(
                [128, 2, BPC, 4, 128], bf16, tag="ctx", name="ctx_mega"
            )
            qry_mega = io.tile([128, BPC, 2, 128], bf16, tag="qry", name="qry_mega")
            qwc_mega = io.tile([128, 2, BPC, 128], bf16, tag="qwc", name="qwc_mega")

            maskC = c16f[:, 0:16]
            maskBD = c16f[:, 16:528]
            ones128 = c128b[:, 0:1]
            rsel4 = c128b[:, 4:8]
            rep4x16 = c16b[:4, 0:16]
            b4x16 = c16b[:, 16:144]

            # ring 1 (sync): ctx h0, qry; ring 2 (scalar): qwc, ctx h1, consts
            nc.sync.dma_start(out=ctx_mega[:, 0], in_=ctx_d[:, 0])
            nc.scalar.dma_start(out=qwc_mega, in_=qwc_d[...])
            nc.scalar.dma_start(out=ctx_mega[:, 1], in_=ctx_d[:, 1])
            nc.sync.dma_start(out=qry_mega, in_=qry_d[...])
            nc.scalar.dma_start(out=sqv, in_=sq_d[...])
            nc.scalar.dma_start(out=c128b, in_=c128b_d[...])
            nc.scalar.dma_start(out=c16f, in_=c16f_d[...])
            nc.scalar.dma_start(out=c16b, in_=c16b_d[...])

            # ---- per-h chain: t[p,(b r)] = sum_d qwc*ctx + sq, then exp.
            # h=0's whole chain is emitted first so DVE never stalls on the
            # h=1 ctx load.
            t_sb = small.tile([128, BPC, 4, 2], fp32, tag="t_sb")
            e32 = small.tile([128, BPC, 4, 2], bf16, tag="e32")
            sqvv = sqv.rearrange("p (b u h) -> p b u h", b=BPC, u=1)
            for h in range(2):
                g = work.tile([128, BPC, 4, 128], bf16, tag=f"g{h}")
                nc.vector.tensor_mul(
                    g,
                    ctx_mega[:, h],
                    qwc_mega[:, h]
                    .rearrange("p b (u d) -> p b u d", u=1)
                    .to_broadcast([128, BPC, 4, 128]),
                )
                s64 = work.tile([128, BPC, 4, 64], bf16, tag=f"s64_{h}")
                nc.vector.tensor_add(s64, g[:, :, :, 0:64], g[:, :, :, 64:128])
                s32 = work.tile([128, BPC, 4, 32], bf16, tag=f"s32_{h}")
                nc.vector.tensor_add(s32, s64[:, :, :, 0:32], s64[:, :, :, 32:64])
                nc.vector.tensor_reduce(
                    out=t_sb[:, :, :, h], in_=s32, axis=X, op=add
                )
                nc.vector.tensor_add(
                    t_sb[:, :, :, h],
                    t_sb[:, :, :, h],
                    sqvv[:, :, :, h].to_broadcast([128, BPC, 4]),
                )
                nc.scalar.activation(
                    out=e32[:, :, :, h], in_=t_sb[:, :, :, h], func=Exp
                )

            e_rh = e32.rearrange("p b r h -> p (b r) h")

            # soft_c denominators as [16,1]: rowsum over j=(p,h) per (b,r)
            rs16_ps = psum.tile([16, 1], fp32, tag="rs16")
            for h in range(2):
                nc.tensor.matmul(
                    rs16_ps, e_rh[:, :, h], ones128,
                    start=(h == 0), stop=(h == 1),
                )
            rec_col = small.tile([16, 1], fp32, tag="rec_col")
            nc.vector.reciprocal(out=rec_col, in_=rs16_ps)

            # soft_q denominators and weights, per h
            u8 = small.tile([128, BPC, 2], fp32, tag="u8")
            recu = small.tile([128, BPC, 2], bf16, tag="recu")
            sqt32 = small.tile([128, BPC, 4, 2], bf16, tag="sqt32")
            for h in range(2):
                nc.vector.tensor_reduce(
                    out=u8[:, :, h], in_=e32[:, :, :, h], axis=X, op=add
                )
                with nc.allow_low_precision(reason="softmax weights bf16"):
                    nc.vector.reciprocal(out=recu[:, :, h], in_=u8[:, :, h])
                nc.vector.tensor_mul(
                    sqt32[:, :, :, h],
                    e32[:, :, :, h],
                    recu[:, :, h]
                    .rearrange("p (b u) -> p b u", u=1)
                    .to_broadcast([128, BPC, 4]),
                )

            sq_flat = sqt32.rearrange("p b r h -> p (b r) h")

            # ---- SM16raw[(b r'), (b r)] = sum_j sqm*e (mask * 1/256 after)
            sm16_ps = psum.tile([16, 16], fp32, tag="sm16")
            for h in range(2):
                nc.tensor.matmul(
                    sm16_ps, sq_flat[:, :, h], e_rh[:, :, h],
                    start=(h == 0), stop=(h == 1),
                )
            sm16 = small.tile([16, 16], bf16, tag="sm16sb")
            nc.vector.tensor_mul(sm16, sm16_ps, maskC)

            # ---- C2Q16raw[(b r), (b d)] = sum_j e * qry
            c2q_ps = psum.tile([16, 512], fp32, tag="c2q")
            c2q_ps_v = c2q_ps.rearrange("m (b d) -> m b d", b=BPC)
            for h in range(2):
                nc.tensor.matmul(
                    c2q_ps_v,
                    e_rh[:, :, h],
                    qry_mega[:, :, h, :],
                    start=(h == 0), stop=(h == 1),
                )
            # c2qm = (c2q_ps * rec_col) * maskBD   (scale + block mask fused)
            c2qm = small.tile([16, 512], bf16, tag="c2qm")
            nc.vector.scalar_tensor_tensor(
                out=c2qm, in0=c2q_ps, scalar=rec_col, in1=maskBD,
                op0=mult, op1=mult,
            )
            nc.scalar.dma_start(out=c2q_d[...], in_=c2qm)

            # ---- CS[r, (b d)] = sum_{p%4=r, h, r'} ctx  (8 accum matmuls)
            cs_ps = psum.tile([4, 512], fp32, tag="cs")
            cs_ps_v = cs_ps.rearrange("m (b d) -> m b d", b=BPC)
            for k in range(8):
                h, r = k // 4, k % 4
                nc.tensor.matmul(
                    cs_ps_v,
                    rsel4,
                    ctx_mega[:, h, :, r, :],
                    start=(k == 0), stop=(k == 7),
                )
            cs4 = small.tile([4, 512], bf16, tag="cs4")
            nc.scalar.copy(out=cs4, in_=cs_ps)

            # ---- CS replicated to (b r') rows, block-diag masked
            csrep_ps = psum.tile([16, 512], fp32, tag="csrep")
            nc.tensor.matmul(csrep_ps, rep4x16, cs4, start=True, stop=True)
            csbd = small.tile([16, 512], bf16, tag="csbd")
            nc.vector.tensor_mul(csbd, csrep_ps, maskBD)

            # ---- Q2C block-diag = sm16M @ csBD (both masked -> no re-mask);
            # soft_c scale folded into the ACT copy
            q2c_ps = psum.tile([16, 512], fp32, tag="q2c")
            nc.tensor.matmul(q2c_ps, sm16, csbd, start=True, stop=True)
            q2cbd = small.tile([16, 512], bf16, tag="q2cbd")
            nc.scalar.activation(out=q2cbd, in_=q2c_ps, func=Copy, scale=rec_col)

            # ---- broadcast rows r -> 128 partitions (p%4 pattern)
            repc_ps = psum.tile([128, 512], fp32, tag="repc")
            nc.tensor.matmul(repc_ps, b4x16, c2qm, start=True, stop=True)
            repc = small.tile([128, 512], bf16, tag="repc")
            nc.scalar.copy(out=repc, in_=repc_ps)
            repq_ps = psum.tile([128, 512], fp32, tag="repq")
            nc.tensor.matmul(repq_ps, b4x16, q2cbd, start=True, stop=True)
            repq = small.tile([128, 512], bf16, tag="repq")
            nc.scalar.copy(out=repq, in_=repq_ps)
            repc_v = repc.rearrange("p (b u d) -> p b u d", b=BPC, u=1)
            repq_v = repq.rearrange("p (b u d) -> p b u d", b=BPC, u=1)

            # ---- products (pre-permuted bf16), all DVE, per h-half with
            # the store DMA issued as soon as each half is ready
            prodc = outp.tile([128, 2, BPC, 4, 128], bf16, tag="prodc")
            prodq = outp.tile([128, 2, BPC, 4, 128], bf16, tag="prodq")
            for h in range(2):
                nc.vector.tensor_mul(
                    prodc[:, h],
                    ctx_mega[:, h],
                    repc_v.to_broadcast([128, BPC, 4, 128]),
                )
                eng = nc.sync if h == 0 else nc.scalar
                eng.dma_start(out=prodc_d[:, h], in_=prodc[:, h])
            for h in range(2):
                nc.vector.tensor_mul(
                    prodq[:, h],
                    ctx_mega[:, h],
                    repq_v.to_broadcast([128, BPC, 4, 128]),
                )
                eng = nc.sync if h == 0 else nc.scalar
                eng.dma_start(out=prodq_d[:, h], in_=prodq[:, h])
    nc.compile()
    return nc


def _get_program():
    global _prog
    if _prog is None:
        _prog = _build_program()
    return _prog


def _make_consts():
    import ml_dtypes

    bf = ml_dtypes.bfloat16
    p = np.arange(128)
    br = np.arange(16)
    c16f = np.zeros((16, 528), np.float32)
    c16f[:, 0:16] = (br[:, None] // 4 == br[None, :] // 4).astype(
        np.float32
    ) / 256.0
    c16f[:, 16:528] = (
        br[:, None] // 4 == np.arange(512)[None, :] // 128
    ).astype(np.float32)
    c128b = np.zeros((128, 8), bf)
    c128b[:, 0] = 1.0
    c128b[:, 4:8] = (p[:, None] % 4 == np.arange(4)[None, :]).astype(bf)
    c16b = np.zeros((16, 144), bf)
    c16b[:4, 0:16] = (np.arange(4)[:, None] == br[None, :] % 4).astype(bf)
    c16b[:, 16:144] = (br[:, None] % 4 == p[None, :] % 4).astype(bf)
    return c16f, c128b, c16b


def _run(context, query, w, trace=False):
    import ml_dtypes
    from concourse.bass_utils import run_bass_kernel_spmd

    bf = ml_dtypes.bfloat16
    nc = _get_program()
    w = np.ascontiguousarray(w, dtype=np.float32)
    w_q, w_c, w_m = w[:D, 0], w[D : 2 * D, 0], w[2 * D :, 0]

    ctx_bf = np.asarray(context, dtype=np.float32).astype(bf)
    qry_bf = np.asarray(query, dtype=np.float32).astype(bf)
    qry32 = qry_bf.astype(np.float32)
    qwc_bf = (qry32 * w_m + w_c).astype(bf)
    sq = (qry32 * w_q).sum(-1)  # (B, 256) fp32

    # ctx[p, h, b, r, d] with n = 256r + 128h + p
    ctx_dev = np.ascontiguousarray(
        ctx_bf.reshape(B, 4, 2, 128, 128).transpose(3, 2, 0, 1, 4)
    )  # (128, 2, B, 4, 128)
    qry_dev = np.ascontiguousarray(
        qry_bf.reshape(B, 2, 128, 128).transpose(2, 0, 1, 3)
    )  # (128, B, 2, 128)
    qwc_dev = np.ascontiguousarray(
        qwc_bf.reshape(B, 2, 128, 128).transpose(2, 1, 0, 3)
    )  # (128, 2, B, 128)
    sq_dev = np.ascontiguousarray(
        sq.reshape(B, 2, 128).transpose(2, 0, 1)
    )  # (128, B, 2)

    c16f, c128b, c16b = _make_consts()
    in_maps = []
    for c in range(NCORES):
        bs = slice(c * BPC, (c + 1) * BPC)
        in_maps.append(
            {
                "ctx": np.ascontiguousarray(ctx_dev[:, :, bs]),
                "qry": np.ascontiguousarray(qry_dev[:, bs]),
                "qwc": np.ascontiguousarray(qwc_dev[:, :, bs]),
                "sqv": np.ascontiguousarray(
                    sq_dev[:, bs].reshape(128, 2 * BPC)
                ),
                "c16f": c16f,
                "c128b": c128b,
                "c16b": c16b,
            }
        )

    res = run_bass_kernel_spmd(
        nc, in_maps, core_ids=list(range(NCORES)), trace=trace
    )

    # ---- host assembly
    out = np.empty((B, N, 4 * D), np.float32)
    out[:, :, 0:128] = context
    c2q_all = np.empty((B, 4, 128), np.float32)
    for c in range(NCORES):
        r = res.results[c]
        c2q = np.asarray(r["c2q"]).astype(np.float32)  # (16, 512)
        for b in range(BPC):
            c2q_all[c * BPC + b] = c2q[4 * b : 4 * b + 4, 128 * b : 128 * b + 128]
        for name, sec in (("prodC", 2), ("prodQ", 3)):
            arr = np.asarray(r[name]).astype(np.float32)  # (128, 2, BPC, 4, 128)
            out[c * BPC : (c + 1) * BPC, :, sec * 128 : sec * 128 + 128] = (
                arr.transpose(2, 3, 1, 0, 4).reshape(BPC, N, 128)
            )
    ridx = np.arange(N) % 4
    out[:, :, 128:256] = c2q_all[:, ridx, :]
    return out, res


def kernel(context, query, c_mask, q_mask, w):
    out, _ = _run(context, query, w, trace=False)
    return out
